# revision 1
# baseline (speedup 1.0000x reference)
"""ControlNorm2D forward on 8 Trainium2 NeuronCores (Bass/Tile).

Reference math (per channel c, batch dim b carries an EMA recurrence):
  mu[b,c]  = mean_{hw} x[b,c,:,:]
  v[b,c]   = var_{hw}  x[b,c,:,:]
  _mu_b    = stale batch-EMA of (m_p, mu, m)      (linear in its 3 inputs)
  var_cur  = v + AFWD*(mu - _mu_b)^2
  _var_b   = stale batch-EMA of (var_p, var_cur, var)
  out      = (x - _mu_b) / sqrt(_var_b + EPS)

The batch-EMA ("lin_momentum" stale output) is a fixed linear map over the
batch dim: stale = Wc^T @ curr + Wp^T @ prev + Ws^T @ stream, with 32x32
matrices built on the host (exact closed form of the conv1d-with-powers
trick, verified against the jax reference).

Sharding: channels C=256 split 8 ways (channel-parallel, no communication).
Per core: x shard [B=32, Csh=32, 4096] = 16 MiB, fully resident in SBUF.
Row (b,c) -> tile t = b%8, partition p = 32*(b//8) + c.  Per tile: DVE
reduce_sum -> sum, ACT Square+accum_out -> sumsq.  The tiny [b,c] stats
block moves between row layout ([128,16], partition=(k,c)) and batch layout
([32,32], partition=b) with a PE transpose plus 0/1 selection-matrix
matmuls (exact in f32) -- compute engines cannot cross partitions, PE can.
EMA applied as accumulating 32x32x32 matmuls.  Normalization: x = S*x + T
in place with per-row scalars, split across ACT (Identity) and DVE
(tensor_scalar), then streamed out.

Instruction-level constraint that shaped the code: a TPB compute
instruction supports only ~1 sync-wait, so dummy 1x1 PE matmuls absorb
DMA/engine semaphores early, each ACT Square gets a private junk output
slot (no WAW waits), and eps/masks come in as host constants.
"""

import numpy as np

B, C, H, W = 32, 256, 64, 64
NCORES = 8
CSH = C // NCORES        # 32 channels per core
FREE = H * W             # 4096
NT = 8                   # row tiles per core: tile t holds b in {t, t+8, t+16, t+24}
AFWD = 0.999
EPS = 1e-5

_CACHE = {}


def _build_ema_weights():
    """stale = Wc^T@curr + Wp^T@prev + Ws^T@stream (float64 math, cast f32).

    new[i] = m^B*stream[i] + (1-m)*( sum_{bb<=i} m^(i-bb) curr[bb]
                                   + sum_{bb>i} m^(B+i-bb) prev[bb] )
    stale[j] = new[j-1] (j>=1);  stale[0] = stream[B-1]
    """
    m = AFWD
    Wc = np.zeros((B, B))
    Wp = np.zeros((B, B))
    Ws = np.zeros((B, B))
    for j in range(1, B):
        i = j - 1
        Ws[i, j] = m ** B
        for bb in range(0, i + 1):
            Wc[bb, j] = (1 - m) * m ** (i - bb)
        for bb in range(i + 1, B):
            Wp[bb, j] = (1 - m) * m ** (B + i - bb)
    Ws[B - 1, 0] = 1.0
    return Wc.astype(np.float32), Wp.astype(np.float32), Ws.astype(np.float32)


def _build_sel_matrices():
    # fan-in: sums = sum_k selA_k^T @ sbT[:, 32k:32k+32], selA_k = selA[:, 32k:*]
    # [16, 32] with selA_k[p, b] = [p == b%8][b//8 == k] (row 8+t of sbT holds
    # sumsq -> selB uses p = 8 + b%8).
    selA = np.zeros((16, 128), np.float32)
    selB = np.zeros((16, 128), np.float32)
    for b in range(B):
        k, t = b // 8, b % 8
        selA[t, 32 * k + b] = 1.0
        selB[8 + t, 32 * k + b] = 1.0
    # fan-out: rows = Sexp^T @ selRT_S + Texp^T @ selRT_T where
    # Sexp[b, 32k+c] = S[b, c]*[b//8==k] (Kmask) and selRT_S[b, t] = [t == b%8].
    kmask = np.zeros((B, 128), np.float32)
    selRT_S = np.zeros((B, 16), np.float32)
    selRT_T = np.zeros((B, 16), np.float32)
    for b in range(B):
        k, t = b // 8, b % 8
        kmask[b, 32 * k:32 * k + 32] = 1.0
        selRT_S[b, t] = 1.0
        selRT_T[b, 8 + t] = 1.0
    return selA, selB, kmask, selRT_S, selRT_T


def _build_module(stages=("pass1", "stats", "stageb", "pass2")):
    import concourse.bass as bass
    import concourse.bacc as bacc
    import concourse.tile as tile
    from concourse import mybir
    from contextlib import ExitStack

    f32 = mybir.dt.float32
    bf16 = mybir.dt.bfloat16
    AF = mybir.ActivationFunctionType
    ALU = mybir.AluOpType

    # Bacc (not raw Bass): its compile() splits multi-sem sync waits into
    # event-semaphore instructions -- TRN2 allows only 1 wait per instruction.
    nc = bacc.Bacc("TRN2", target_bir_lowering=False, debug=False)

    x_in = nc.dram_tensor("x", [B, CSH, FREE], f32, kind="ExternalInput").ap()
    out_d = nc.dram_tensor("out", [B, CSH, FREE], f32, kind="ExternalOutput").ap()
    mst_d = nc.dram_tensor("mst", [B, CSH], f32, kind="ExternalInput").ap()
    vst_d = nc.dram_tensor("vst", [B, CSH], f32, kind="ExternalInput").ap()
    mp_d = nc.dram_tensor("mp", [B, CSH], f32, kind="ExternalInput").ap()
    vp_d = nc.dram_tensor("vp", [B, CSH], f32, kind="ExternalInput").ap()
    wc_d = nc.dram_tensor("wc", [B, B], f32, kind="ExternalInput").ap()
    wp_d = nc.dram_tensor("wp", [B, B], f32, kind="ExternalInput").ap()
    ws_d = nc.dram_tensor("ws", [B, B], f32, kind="ExternalInput").ap()
    id_d = nc.dram_tensor("ident", [128, 128], f32, kind="ExternalInput").ap()
    selA_d = nc.dram_tensor("selA", [16, 128], f32, kind="ExternalInput").ap()
    selB_d = nc.dram_tensor("selB", [16, 128], f32, kind="ExternalInput").ap()
    km_d = nc.dram_tensor("kmask", [B, 128], f32, kind="ExternalInput").ap()
    rtS_d = nc.dram_tensor("selRT_S", [B, 16], f32, kind="ExternalInput").ap()
    rtT_d = nc.dram_tensor("selRT_T", [B, 16], f32, kind="ExternalInput").ap()
    eps_d = nc.dram_tensor("epsv", [B, 1], f32, kind="ExternalInput").ap()

    with tile.TileContext(nc) as tc, ExitStack() as ctx:
        xp = ctx.enter_context(tc.tile_pool(name="xp", bufs=NT))
        jp = ctx.enter_context(tc.tile_pool(name="jp", bufs=NT))
        cons = ctx.enter_context(tc.tile_pool(name="cons", bufs=1))
        sm = ctx.enter_context(tc.tile_pool(name="sm", bufs=1))
        pp = ctx.enter_context(tc.tile_pool(name="pp", bufs=1, space="PSUM"))

        def load_const(name, shape, dram_ap):
            t = cons.tile(shape, f32, tag=name)
            nc.sync.dma_start(t[:], dram_ap)
            return t

        ident = load_const("ident", [128, 128], id_d)
        wc = load_const("wc", [B, B], wc_d)
        wp = load_const("wp", [B, B], wp_d)
        ws = load_const("ws", [B, B], ws_d)
        mst = load_const("mst", [B, CSH], mst_d)
        vst = load_const("vst", [B, CSH], vst_d)
        mp = load_const("mp", [B, CSH], mp_d)
        vp = load_const("vp", [B, CSH], vp_d)
        selA = load_const("selA", [16, 128], selA_d)
        selB = load_const("selB", [16, 128], selB_d)
        kmask = load_const("kmask", [B, 128], km_d)
        selRT_S = load_const("selRT_S", [B, 16], rtS_d)
        selRT_T = load_const("selRT_T", [B, 16], rtT_d)
        eps = load_const("epsv", [B, 1], eps_d)

        # ACT table warmup (Square/Sqrt/Identity share one ACT table set)
        warm = cons.tile([1, 1], f32, tag="warm")
        nc.vector.memset(warm[:], 1.0)
        nc.scalar.activation(warm[:], warm[:], AF.Square)

        # Dummy 1x1 matmuls (one accumulation group) so the PE observes every
        # constant-DMA semaphore early -- compute instructions only support a
        # single sync-wait, so the real matmuls must not face >1 new condition.
        consts = [ident, wc, wp, ws, mst, vst, mp, vp, selA, selB, kmask,
                  selRT_S, selRT_T, eps]
        jps = pp.tile([1, 1], f32, tag="jps")
        for i, cst in enumerate(consts):
            nc.tensor.matmul(jps[:], cst[:1, :1], cst[:1, :1],
                             start=(i == 0), stop=(i == len(consts) - 1))

        # pass 1: load x tiles; per-row sum (DVE) and sumsq (ACT, private junk
        # slot per tile to avoid WAW waits)
        stats = sm.tile([128, 16], f32, tag="stats")  # col t: sum, 8+t: sumsq
        xts = []
        junks = []
        for t in range(NT):
            xt = xp.tile([128, FREE], f32, tag="x")
            xts.append(xt)
            nc.sync.dma_start(xt[:], x_in[t::NT])
            if "stats" in stages:
                nc.vector.reduce_sum(stats[:, t:t + 1], xt[:], axis=mybir.AxisListType.X)
                junk = jp.tile([128, FREE], bf16, tag="junk")
                junks.append(junk)
                nc.scalar.activation(junk[:], xt[:], AF.Square,
                                     accum_out=stats[:, 8 + t:9 + t])

        if "stageb" in stages:
            # absorb the ACT semaphore on PE before the stats transpose (which
            # would otherwise need to wait on both DVE and ACT)
            jps2 = pp.tile([1, 1], f32, tag="jps2")
            nc.tensor.matmul(jps2[:], junks[-1][:1, :1], junks[-1][:1, :1],
                             start=True, stop=True)

            # stage B: stats -> batch layout [32b, 32c] (transpose + selection mm)
            psT = pp.tile([16, 128], f32, tag="psT")
            nc.tensor.transpose(psT[:], stats[:], ident[:])
            sbT = sm.tile([16, 128], f32, tag="sbT")
            nc.vector.tensor_copy(sbT[:], psT[:])
            pSums = pp.tile([B, CSH], f32, tag="pSums")
            pSq = pp.tile([B, CSH], f32, tag="pSq")
            for k in range(4):
                nc.tensor.matmul(pSums[:], selA[:, 32 * k:32 * k + 32],
                                 sbT[:, 32 * k:32 * k + 32],
                                 start=(k == 0), stop=(k == 3))
            for k in range(4):
                nc.tensor.matmul(pSq[:], selB[:, 32 * k:32 * k + 32],
                                 sbT[:, 32 * k:32 * k + 32],
                                 start=(k == 0), stop=(k == 3))

            rN = float(1.0 / FREE)
            mu = sm.tile([B, CSH], f32, tag="mu")
            nc.vector.tensor_scalar_mul(mu[:], pSums[:], rN)
            sqs = sm.tile([B, CSH], f32, tag="sqs")
            nc.vector.tensor_scalar_mul(sqs[:], pSq[:], rN)
            musq = sm.tile([B, CSH], f32, tag="musq")
            nc.vector.tensor_mul(musq[:], mu[:], mu[:])
            v = sm.tile([B, CSH], f32, tag="v")  # v = sq/N - mu^2
            nc.vector.tensor_sub(v[:], sqs[:], musq[:])

            # _mu_b = Wc^T@mu + Wp^T@mp + Ws^T@mst
            pmu = pp.tile([B, CSH], f32, tag="pmu")
            nc.tensor.matmul(pmu[:], wc[:], mu[:], start=True, stop=False)
            nc.tensor.matmul(pmu[:], wp[:], mp[:], start=False, stop=False)
            nc.tensor.matmul(pmu[:], ws[:], mst[:], start=False, stop=True)
            mub = sm.tile([B, CSH], f32, tag="mub")
            nc.vector.tensor_copy(mub[:], pmu[:])

            d = sm.tile([B, CSH], f32, tag="d")
            nc.vector.tensor_sub(d[:], mu[:], mub[:])
            d2 = sm.tile([B, CSH], f32, tag="d2")
            nc.vector.tensor_mul(d2[:], d[:], d[:])
            vc = sm.tile([B, CSH], f32, tag="vc")  # var_cur = AFWD*d2 + v
            nc.vector.scalar_tensor_tensor(vc[:], d2[:], float(AFWD), v[:],
                                           op0=ALU.mult, op1=ALU.add)

            # _var_b = Wc^T@vc + Wp^T@vp + Ws^T@vst
            pvar = pp.tile([B, CSH], f32, tag="pvar")
            nc.tensor.matmul(pvar[:], wc[:], vc[:], start=True, stop=False)
            nc.tensor.matmul(pvar[:], wp[:], vp[:], start=False, stop=False)
            nc.tensor.matmul(pvar[:], ws[:], vst[:], start=False, stop=True)

            std = sm.tile([B, CSH], f32, tag="std")
            nc.scalar.activation(std[:], pvar[:], AF.Sqrt, bias=eps[:])
            S = sm.tile([B, CSH], f32, tag="S")
            nc.vector.reciprocal(S[:], std[:])
            T = sm.tile([B, CSH], f32, tag="T")  # T = -mub * S
            nc.vector.scalar_tensor_tensor(T[:], mub[:], -1.0, S[:],
                                           op0=ALU.mult, op1=ALU.mult)

            # back to row layout: rows[32k+c, t] = S[8k+t, c], col 8+t same for T.
            # Sexp[b, 32k+c] = S[b,c]*[b//8==k] (broadcast * kmask), then one
            # accumulating matmul pair: rows_ps = Sexp^T@selRT_S + Texp^T@selRT_T.
            Sexp = sm.tile([B, 128], f32, tag="Sexp")
            nc.vector.tensor_tensor(
                out=Sexp[:].rearrange("p (a b) -> p a b", a=4),
                in0=S[:].unsqueeze(1).broadcast_to((B, 4, CSH)),
                in1=kmask[:].rearrange("p (a b) -> p a b", a=4),
                op=ALU.mult)
            Texp = sm.tile([B, 128], f32, tag="Texp")
            nc.vector.tensor_tensor(
                out=Texp[:].rearrange("p (a b) -> p a b", a=4),
                in0=T[:].unsqueeze(1).broadcast_to((B, 4, CSH)),
                in1=kmask[:].rearrange("p (a b) -> p a b", a=4),
                op=ALU.mult)
            rows_ps = pp.tile([128, 16], f32, tag="rows_ps")
            nc.tensor.matmul(rows_ps[:], Sexp[:], selRT_S[:], start=True, stop=False)
            nc.tensor.matmul(rows_ps[:], Texp[:], selRT_T[:], start=False, stop=True)
            rows = sm.tile([128, 16], f32, tag="rows")
            nc.vector.tensor_copy(rows[:], rows_ps[:])

            # absorb the DVE(rows) semaphore on ACT so each in-place pass-2
            # activation needs only its single WAR self-wait
            warm2 = cons.tile([1, 1], f32, tag="warm2")
            nc.scalar.activation(warm2[:], rows[:1, :1], AF.Square)

        if "pass2" in stages:
            # pass 2: x = S*x + T in place, tiles split across ACT and DVE
            for t in range(NT):
                if t % 2 == 0:
                    nc.scalar.activation(xts[t][:], xts[t][:], AF.Identity,
                                         bias=rows[:, 8 + t:9 + t],
                                         scale=rows[:, t:t + 1])
                else:
                    nc.vector.tensor_scalar(xts[t][:], xts[t][:],
                                            rows[:, t:t + 1], rows[:, 8 + t:9 + t],
                                            op0=ALU.mult, op1=ALU.add)
                nc.gpsimd.dma_start(out_d[t::NT], xts[t][:])

    nc.compile()
    return nc


def _get_module():
    if "nc" not in _CACHE:
        _CACHE["nc"] = _build_module()
    return _CACHE["nc"]


def kernel(x, m, var, m_p, var_p, u, u_p, v_p, beta_p, alpha_p):
    from concourse.bass_utils import run_bass_kernel_spmd

    nc = _get_module()
    Wc, Wp, Ws = _build_ema_weights()
    selA, selB, kmask, selRT_S, selRT_T = _build_sel_matrices()
    ident = np.eye(128, dtype=np.float32)
    epsv = np.full((B, 1), EPS, np.float32)

    x = np.asarray(x, dtype=np.float32)
    m = np.asarray(m, dtype=np.float32)
    var = np.asarray(var, dtype=np.float32)
    m_p = np.asarray(m_p, dtype=np.float32)
    var_p = np.asarray(var_p, dtype=np.float32)

    x4 = x.reshape(B, C, FREE)
    in_maps = []
    for i in range(NCORES):
        cs = slice(i * CSH, (i + 1) * CSH)
        in_maps.append({
            "x": np.ascontiguousarray(x4[:, cs, :]),
            "mst": np.ascontiguousarray(m[:, cs]),
            "vst": np.ascontiguousarray(var[:, cs]),
            "mp": np.ascontiguousarray(m_p[:, cs]),
            "vp": np.ascontiguousarray(var_p[:, cs]),
            "wc": Wc, "wp": Wp, "ws": Ws, "ident": ident,
            "selA": selA, "selB": selB, "kmask": kmask,
            "selRT_S": selRT_S, "selRT_T": selRT_T, "epsv": epsv,
        })

    res = run_bass_kernel_spmd(nc, in_maps, list(range(NCORES)),
                               **_CACHE.get("run_kwargs", {}))
    _CACHE["last_results"] = res
    out = np.empty((B, C, FREE), dtype=np.float32)
    for i in range(NCORES):
        out[:, i * CSH:(i + 1) * CSH, :] = res.results[i]["out"]
    return out.reshape(B, C, H, W)



# revision 3
# speedup vs baseline: 1.5315x; 1.5315x over previous
"""ControlNorm2D forward on 8 Trainium2 NeuronCores (Bass/Tile), streaming.

Reference math (per channel c, batch dim b carries an EMA recurrence):
  mu[b,c]  = mean_{hw} x[b,c,:,:]
  v[b,c]   = var_{hw}  x[b,c,:,:]
  _mu_b    = stale batch-EMA of (m_p, mu, m)      (linear in its 3 inputs)
  var_cur  = v + AFWD*(mu - _mu_b)^2
  _var_b   = stale batch-EMA of (var_p, var_cur, var)
  out      = (x - _mu_b) / sqrt(_var_b + EPS)

Key structural facts exploited here:
 1. The EMA is *causal* in the batch dim: output batch b needs stats of
    batches < b only.  So the kernel streams groups of 4 batches
    (4 x 32ch = 128 partitions): load -> stats -> group EMA (carry from
    previous group) -> normalize -> store, letting output DMA overlap
    input DMA.  All DMA transfers serialize on the shared DMA-engine
    bundle (360 GB/s), so overlap + fewer bytes is the whole game.
 2. new[i] = m*new[i-1] + (1-m)*curr[i] + inj[i], where inj[] depends
    only on the small control inputs (m/var/m_p/var_p) -- precomputed on
    the HOST and folded (with eps and group-carry corrections) into
    per-group bias tables.  On-device the recurrence is 3 tiny matmuls
    per group per chain (stale from in-group stats, stale from carry,
    carry update), channel-block-diagonal 128x128 stationaries.
 3. Output is stored as bf16 (tolerance is 2e-2; bf16 rounds at ~4e-3)
    and converted back to f32 on the host: output DMA bytes halve.

Per core per group g (partition p = 32*j + c, j = batch-in-group):
  s = rowsum(x_g)  [DVE]        q = rowsumsq(x_g)  [ACT Square+accum]
  P_smu  = AS^T s + C^T ymu_{g-1}   [PE, PSUM]
  smu    = P_smu + KMUS[:,g]        [ACT Identity+bias]   (stale mean)
  ymu_g  = m^4 ymu_{g-1} + A3S^T s  [PE + DVE stt]        (carry)
  e      = s/N - smu; vc = AFWD*e^2 + (q/N - (s/N)^2)     [DVE+ACT]
  P_svar = AV^T vc + C^T yvar_{g-1} [PE]
  std    = sqrt(P_svar + KVARSE[:,g])  [ACT, eps folded]
  yvar_g = m^4 yvar_{g-1} + A3V^T vc
  S = 1/std [DVE]; T = -smu*S [DVE]
  out_g  = S*x_g + T  -> bf16   [ACT/DVE alternating]  -> store [Pool DMA]

Sharding: channels C=256 split 8 ways (channel-parallel, no comms).
"""

import numpy as np

B, C, H, W = 32, 256, 64, 64
NCORES = 8
CSH = C // NCORES        # 32 channels per core
FREE = H * W             # 4096
G = 8                    # batch groups per core
GB = 4                   # batches per group (4*32ch = 128 partitions)
AFWD = 0.999
EPS = 1e-5
RN = 1.0 / FREE
NCONST = 657             # 5 matrices (5*128) + KMUS(8) + KVARSE(8) + zero col

_CACHE = {}

# groups whose normalize runs on DVE (rest on ACT); late groups stay on ACT
DVE_NORM_GROUPS = (0, 2, 4)


def _build_matrices():
    """Channel-block-diagonal recurrence matrices, [128,128] f32.

    AS:  stale-from-in-group-stats, 1/N folded (consumes raw row sums)
    AV:  same in variance units (consumes var_cur directly)
    A3S/A3V: carry update (y = new[last batch of group])
    C:   stale-from-carry (reads partition slots 96+c only)
    """
    m = np.float64(AFWD)
    AS = np.zeros((128, 128))
    AV = np.zeros((128, 128))
    A3S = np.zeros((128, 128))
    A3V = np.zeros((128, 128))
    Cm = np.zeros((128, 128))
    for c in range(CSH):
        for j in range(GB):
            for jp in range(j):
                AV[32 * jp + c, 32 * j + c] = (1 - m) * m ** (j - 1 - jp)
                AS[32 * jp + c, 32 * j + c] = (1 - m) * m ** (j - 1 - jp) * RN
            Cm[96 + c, 32 * j + c] = m ** j
        for jp in range(GB):
            A3V[32 * jp + c, 96 + c] = (1 - m) * m ** (3 - jp)
            A3S[32 * jp + c, 96 + c] = (1 - m) * m ** (3 - jp) * RN
    return AS, AV, A3S, A3V, Cm


def _build_inj(stream, prev, m):
    # inj for new[i] = m*new[i-1] + (1-m)*curr[i] + inj[i], new[-1] = 0
    Bn = stream.shape[0]
    inj = np.zeros_like(stream)
    mB = m ** Bn
    inj[0] = mB * stream[0] + (1 - m) * sum(
        m ** (Bn - pi) * prev[pi] for pi in range(1, Bn))
    for i in range(1, Bn):
        inj[i] = mB * (stream[i] - m * stream[i - 1]) - (1 - m) * mB * prev[i]
    return inj


def _bias_table(inj, g0_stream_last):
    """[128, G] stale-bias table; inj terms + carry (k3) deficit corrections."""
    m = np.float64(AFWD)
    c_all = np.arange(CSH)
    K = np.zeros((128, G))
    k3cum = np.zeros(CSH)
    for g in range(G):
        for j in range(GB):
            K[32 * j + c_all, g] = sum(
                m ** (j - 1 - jp) * inj[4 * g + jp] for jp in range(j))
            K[32 * j + c_all, g] += m ** j * k3cum
        k3 = sum(m ** (3 - jp) * inj[4 * g + jp] for jp in range(GB))
        k3cum = m ** 4 * k3cum + k3
    K[c_all, 0] += g0_stream_last  # stale[0] = stream[B-1]
    return K


def _build_const_block(m_in, var_in, mp, vp):
    """Pack everything into one [128, NCONST] f32 tensor (one DMA)."""
    m = np.float64(AFWD)
    if "mats" not in _CACHE:
        _CACHE["mats"] = _build_matrices()
    AS, AV, A3S, A3V, Cm = _CACHE["mats"]
    inj_mu = _build_inj(m_in.astype(np.float64), mp.astype(np.float64), m)
    inj_var = _build_inj(var_in.astype(np.float64), vp.astype(np.float64), m)
    KMUS = _bias_table(inj_mu, m_in[B - 1].astype(np.float64))
    KVARSE = _bias_table(inj_var, var_in[B - 1].astype(np.float64)) + EPS
    cst = np.zeros((128, NCONST), np.float32)
    cst[:, 0:128] = AS
    cst[:, 128:256] = AV
    cst[:, 256:384] = A3S
    cst[:, 384:512] = A3V
    cst[:, 512:640] = Cm
    cst[:, 640:648] = KMUS
    cst[:, 648:656] = KVARSE
    # col 656 stays zero: initial carry y_{-1}
    return cst


def _build_module():
    import concourse.bass as bass
    import concourse.bacc as bacc
    import concourse.tile as tile
    from concourse import mybir
    from contextlib import ExitStack

    f32 = mybir.dt.float32
    bf16 = mybir.dt.bfloat16
    AF = mybir.ActivationFunctionType
    ALU = mybir.AluOpType
    X = mybir.AxisListType.X
    M4 = float(AFWD) ** 4

    nc = bacc.Bacc("TRN2", target_bir_lowering=False, debug=False)

    x_in = nc.dram_tensor("x", [B, CSH, FREE], f32, kind="ExternalInput").ap()
    out_d = nc.dram_tensor("out", [B, CSH, FREE], bf16, kind="ExternalOutput").ap()
    cst_d = nc.dram_tensor("cst", [128, NCONST], f32, kind="ExternalInput").ap()

    with tile.TileContext(nc) as tc, ExitStack() as ctx:
        xp = ctx.enter_context(tc.tile_pool(name="xp", bufs=5))
        op = ctx.enter_context(tc.tile_pool(name="op", bufs=G))
        jp = ctx.enter_context(tc.tile_pool(name="jp", bufs=2))
        cons = ctx.enter_context(tc.tile_pool(name="cons", bufs=1))
        sm = ctx.enter_context(tc.tile_pool(name="sm", bufs=1))
        pp = ctx.enter_context(tc.tile_pool(name="pp", bufs=2, space="PSUM"))

        # one packed const DMA on the ACT queue (SP queue stays clear for x)
        cst = cons.tile([128, NCONST], f32, tag="cst")
        nc.scalar.dma_start(cst[:], cst_d)
        AS = cst[:, 0:128]
        AV = cst[:, 128:256]
        A3S = cst[:, 256:384]
        A3V = cst[:, 384:512]
        Cm = cst[:, 512:640]
        KMUS = cst[:, 640:648]
        KVARSE = cst[:, 648:656]
        ZERO = cst[:, 656:657]

        # ACT table warmup: Sqrt selects a table set that also serves
        # Square/Identity -- one load, no switches later.
        warm = cons.tile([1, 1], f32, tag="warm")
        nc.vector.memset(warm[:], 1.0)
        nc.scalar.activation(warm[:], warm[:], AF.Sqrt)

        ymu_prev = ZERO
        yvar_prev = ZERO
        for g in range(G):
            xt = xp.tile([128, FREE], f32, tag="x")
            nc.sync.dma_start(xt[:], x_in[GB * g:GB * g + GB])

            s = sm.tile([128, 1], f32, tag=f"s{g}")
            nc.vector.reduce_sum(s[:], xt[:], axis=X)
            junk = jp.tile([128, FREE], bf16, tag="junk")
            q = sm.tile([128, 1], f32, tag=f"q{g}")
            nc.scalar.activation(junk[:], xt[:], AF.Square, accum_out=q[:])

            p_smu = pp.tile([128, 1], f32, tag="psmu")
            nc.tensor.matmul(p_smu[:], AS, s[:], start=True, stop=False)
            nc.tensor.matmul(p_smu[:], Cm, ymu_prev, start=False, stop=True)
            p_ymu = pp.tile([128, 1], f32, tag="pymu")
            nc.tensor.matmul(p_ymu[:], A3S, s[:], start=True, stop=True)

            smu = sm.tile([128, 1], f32, tag=f"smu{g}")
            nc.scalar.activation(smu[:], p_smu[:], AF.Identity,
                                 bias=KMUS[:, g:g + 1])
            ymu = sm.tile([128, 1], f32, tag=f"ymu{g}")
            nc.vector.scalar_tensor_tensor(ymu[:], ymu_prev, M4, p_ymu[:],
                                           op0=ALU.mult, op1=ALU.add)

            e = sm.tile([128, 1], f32, tag=f"e{g}")
            nc.vector.scalar_tensor_tensor(e[:], s[:], RN, smu[:],
                                           op0=ALU.mult, op1=ALU.subtract)
            musq = sm.tile([128, 1], f32, tag=f"musq{g}")
            nc.scalar.activation(musq[:], s[:], AF.Square, scale=RN)
            e2 = sm.tile([128, 1], f32, tag=f"e2{g}")
            nc.scalar.activation(e2[:], e[:], AF.Square)
            vpr = sm.tile([128, 1], f32, tag=f"vpr{g}")
            nc.vector.scalar_tensor_tensor(vpr[:], q[:], RN, musq[:],
                                           op0=ALU.mult, op1=ALU.subtract)
            vc = sm.tile([128, 1], f32, tag=f"vc{g}")
            nc.vector.scalar_tensor_tensor(vc[:], e2[:], float(AFWD), vpr[:],
                                           op0=ALU.mult, op1=ALU.add)

            p_svar = pp.tile([128, 1], f32, tag="psvar")
            nc.tensor.matmul(p_svar[:], AV, vc[:], start=True, stop=False)
            nc.tensor.matmul(p_svar[:], Cm, yvar_prev, start=False, stop=True)
            p_yvar = pp.tile([128, 1], f32, tag="pyvar")
            nc.tensor.matmul(p_yvar[:], A3V, vc[:], start=True, stop=True)

            yvar = sm.tile([128, 1], f32, tag=f"yvar{g}")
            nc.vector.scalar_tensor_tensor(yvar[:], yvar_prev, M4, p_yvar[:],
                                           op0=ALU.mult, op1=ALU.add)
            std = sm.tile([128, 1], f32, tag=f"std{g}")
            nc.scalar.activation(std[:], p_svar[:], AF.Sqrt,
                                 bias=KVARSE[:, g:g + 1])
            Sg = sm.tile([128, 1], f32, tag=f"S{g}")
            nc.vector.reciprocal(Sg[:], std[:])
            Tg = sm.tile([128, 1], f32, tag=f"T{g}")
            nc.vector.scalar_tensor_tensor(Tg[:], smu[:], -1.0, Sg[:],
                                           op0=ALU.mult, op1=ALU.mult)

            outt = op.tile([128, FREE], bf16, tag="out")
            if g in DVE_NORM_GROUPS:
                nc.vector.tensor_scalar(outt[:], xt[:], Sg[:], Tg[:],
                                        op0=ALU.mult, op1=ALU.add)
            else:
                nc.scalar.activation(outt[:], xt[:], AF.Identity,
                                     bias=Tg[:], scale=Sg[:])
            nc.gpsimd.dma_start(out_d[GB * g:GB * g + GB], outt[:])

            ymu_prev = ymu[:]
            yvar_prev = yvar[:]

    nc.compile()
    return nc


def _get_module():
    if "nc" not in _CACHE:
        _CACHE["nc"] = _build_module()
    return _CACHE["nc"]


def kernel(x, m, var, m_p, var_p, u, u_p, v_p, beta_p, alpha_p):
    from concourse.bass_utils import run_bass_kernel_spmd

    nc = _get_module()

    x = np.asarray(x, dtype=np.float32)
    m = np.asarray(m, dtype=np.float32)
    var = np.asarray(var, dtype=np.float32)
    m_p = np.asarray(m_p, dtype=np.float32)
    var_p = np.asarray(var_p, dtype=np.float32)

    x4 = x.reshape(B, C, FREE)
    in_maps = []
    for i in range(NCORES):
        cs = slice(i * CSH, (i + 1) * CSH)
        in_maps.append({
            "x": np.ascontiguousarray(x4[:, cs, :]),
            "cst": _build_const_block(m[:, cs], var[:, cs],
                                      m_p[:, cs], var_p[:, cs]),
        })

    res = run_bass_kernel_spmd(nc, in_maps, list(range(NCORES)),
                               **_CACHE.get("run_kwargs", {}))
    _CACHE["last_results"] = res
    out = np.empty((B, C, FREE), dtype=np.float32)
    for i in range(NCORES):
        out[:, i * CSH:(i + 1) * CSH, :] = np.asarray(
            res.results[i]["out"]).astype(np.float32)
    return out.reshape(B, C, H, W)


# revision 12
# speedup vs baseline: 1.6149x; 1.0544x over previous
"""ControlNorm2D forward on 8 Trainium2 NeuronCores (Bass/Tile), streaming.

Reference math (per channel c, batch dim b carries an EMA recurrence):
  mu[b,c]  = mean_{hw} x[b,c,:,:]
  v[b,c]   = var_{hw}  x[b,c,:,:]
  _mu_b    = stale batch-EMA of (m_p, mu, m)      (linear in its 3 inputs)
  var_cur  = v + AFWD*(mu - _mu_b)^2
  _var_b   = stale batch-EMA of (var_p, var_cur, var)
  out      = (x - _mu_b) / sqrt(_var_b + EPS)

Key structural facts exploited here:
 1. The EMA is *causal* in the batch dim: output batch b needs stats of
    batches < b only.  So the kernel streams groups of 4 batches
    (4 x 32ch = 128 partitions): load -> stats -> group EMA (carry from
    previous group) -> normalize -> store, letting output DMA overlap
    input DMA.  All DMA transfers serialize on the shared DMA-engine
    bundle (360 GB/s), so overlap + fewer bytes is the whole game.
 2. new[i] = m*new[i-1] + (1-m)*curr[i] + inj[i], where inj[] depends
    only on the small control inputs (m/var/m_p/var_p) -- precomputed on
    the HOST and folded (with eps and group-carry corrections) into
    per-group bias tables.  On-device the recurrence is 3 tiny matmuls
    per group per chain (stale from in-group stats, stale from carry,
    carry update), channel-block-diagonal 128x128 stationaries.
 3. Output is stored as bf16 (tolerance is 2e-2; bf16 rounds at ~4e-3)
    and converted back to f32 on the host: output DMA bytes halve.

Per core per group g (partition p = 32*j + c, j = batch-in-group):
  s = rowsum(x_g)  [DVE]        q = rowsumsq(x_g)  [ACT Square+accum]
  P_smu  = AS^T s + C^T ymu_{g-1}   [PE, PSUM]
  smu    = P_smu + KMUS[:,g]        [ACT Identity+bias]   (stale mean)
  ymu_g  = m^4 ymu_{g-1} + A3S^T s  [PE + DVE stt]        (carry)
  e      = s/N - smu; vc = AFWD*e^2 + (q/N - (s/N)^2)     [DVE+ACT]
  P_svar = AV^T vc + C^T yvar_{g-1} [PE]
  std    = sqrt(P_svar + KVARSE[:,g])  [ACT, eps folded]
  yvar_g = m^4 yvar_{g-1} + A3V^T vc
  S = 1/std [DVE]; T = -smu*S [DVE]
  out_g  = S*x_g + T  -> bf16   [ACT/DVE alternating]  -> store [Pool DMA]

Sharding: channels C=256 split 8 ways (channel-parallel, no comms).
"""

import numpy as np

B, C, H, W = 32, 256, 64, 64
NCORES = 8
CSH = C // NCORES        # 32 channels per core
FREE = H * W             # 4096
G = 8                    # batch groups per core
GB = 4                   # batches per group (4*32ch = 128 partitions)
AFWD = 0.999
EPS = 1e-5
RN = 1.0 / FREE
NCONST = 657             # 5 matrices (5*128) + KMUS(8) + KVARSE(8) + zero col

_CACHE = {}

# normalize engine assignment: full-tile on ACT for early groups; late
# groups split into ACT half + DVE half (with split stores) so the tail
# stores are ready the moment the DMA engines free up
DVE_NORM_GROUPS = ()
SPLIT_NORM_GROUPS = (4, 5, 6, 7)


def _build_matrices():
    """Channel-block-diagonal recurrence matrices, [128,128] f32.

    AS:  stale-from-in-group-stats, 1/N folded (consumes raw row sums)
    AV:  same in variance units (consumes var_cur directly)
    A3S/A3V: carry update (y = new[last batch of group])
    C:   stale-from-carry (reads partition slots 96+c only)
    """
    m = np.float64(AFWD)
    AS = np.zeros((128, 128))
    AV = np.zeros((128, 128))
    A3S = np.zeros((128, 128))
    A3V = np.zeros((128, 128))
    Cm = np.zeros((128, 128))
    for c in range(CSH):
        for j in range(GB):
            for jp in range(j):
                AV[32 * jp + c, 32 * j + c] = (1 - m) * m ** (j - 1 - jp)
                AS[32 * jp + c, 32 * j + c] = (1 - m) * m ** (j - 1 - jp) * RN
            Cm[96 + c, 32 * j + c] = m ** j
        for jp in range(GB):
            A3V[32 * jp + c, 96 + c] = (1 - m) * m ** (3 - jp)
            A3S[32 * jp + c, 96 + c] = (1 - m) * m ** (3 - jp) * RN
    return AS, AV, A3S, A3V, Cm


def _build_inj(stream, prev, m):
    # inj for new[i] = m*new[i-1] + (1-m)*curr[i] + inj[i], new[-1] = 0
    Bn = stream.shape[0]
    inj = np.zeros_like(stream)
    mB = m ** Bn
    inj[0] = mB * stream[0] + (1 - m) * sum(
        m ** (Bn - pi) * prev[pi] for pi in range(1, Bn))
    for i in range(1, Bn):
        inj[i] = mB * (stream[i] - m * stream[i - 1]) - (1 - m) * mB * prev[i]
    return inj


def _bias_table(inj, g0_stream_last):
    """[128, G] stale-bias table; inj terms + carry (k3) deficit corrections."""
    m = np.float64(AFWD)
    c_all = np.arange(CSH)
    K = np.zeros((128, G))
    k3cum = np.zeros(CSH)
    for g in range(G):
        for j in range(GB):
            K[32 * j + c_all, g] = sum(
                m ** (j - 1 - jp) * inj[4 * g + jp] for jp in range(j))
            K[32 * j + c_all, g] += m ** j * k3cum
        k3 = sum(m ** (3 - jp) * inj[4 * g + jp] for jp in range(GB))
        k3cum = m ** 4 * k3cum + k3
    K[c_all, 0] += g0_stream_last  # stale[0] = stream[B-1]
    return K


def _build_const_block(m_in, var_in, mp, vp):
    """Pack everything into one [128, NCONST] f32 tensor (one DMA)."""
    m = np.float64(AFWD)
    if "mats" not in _CACHE:
        _CACHE["mats"] = _build_matrices()
    AS, AV, A3S, A3V, Cm = _CACHE["mats"]
    inj_mu = _build_inj(m_in.astype(np.float64), mp.astype(np.float64), m)
    inj_var = _build_inj(var_in.astype(np.float64), vp.astype(np.float64), m)
    KMUS = _bias_table(inj_mu, m_in[B - 1].astype(np.float64))
    KVARSE = _bias_table(inj_var, var_in[B - 1].astype(np.float64)) + EPS
    cst = np.zeros((128, NCONST), np.float32)
    cst[:, 0:128] = AS
    cst[:, 128:256] = AV
    cst[:, 256:384] = A3S
    cst[:, 384:512] = A3V
    cst[:, 512:640] = Cm
    cst[:, 640:648] = KMUS
    cst[:, 648:656] = KVARSE
    # col 656 stays zero: initial carry y_{-1}
    return cst


def _build_module():
    import concourse.bass as bass
    import concourse.bacc as bacc
    import concourse.tile as tile
    from concourse import mybir
    from contextlib import ExitStack

    f32 = mybir.dt.float32
    bf16 = mybir.dt.bfloat16
    AF = mybir.ActivationFunctionType
    ALU = mybir.AluOpType
    X = mybir.AxisListType.X
    M4 = float(AFWD) ** 4

    nc = bacc.Bacc("TRN2", target_bir_lowering=False, debug=False)

    x_in = nc.dram_tensor("x", [B, CSH, FREE], f32, kind="ExternalInput").ap()
    out_d = nc.dram_tensor("out", [B, CSH, FREE], bf16, kind="ExternalOutput").ap()
    cst_d = nc.dram_tensor("cst", [128, NCONST], f32, kind="ExternalInput").ap()

    with tile.TileContext(nc) as tc, ExitStack() as ctx:
        xp = ctx.enter_context(tc.tile_pool(name="xp", bufs=5))
        op = ctx.enter_context(tc.tile_pool(name="op", bufs=G))
        jp = ctx.enter_context(tc.tile_pool(name="jp", bufs=2))
        cons = ctx.enter_context(tc.tile_pool(name="cons", bufs=1))
        sm = ctx.enter_context(tc.tile_pool(name="sm", bufs=1))
        pp = ctx.enter_context(tc.tile_pool(name="pp", bufs=2, space="PSUM"))

        # one packed const DMA on the ACT queue (SP queue stays clear for x)
        cst = cons.tile([128, NCONST], f32, tag="cst")
        nc.scalar.dma_start(cst[:], cst_d)
        AS = cst[:, 0:128]
        AV = cst[:, 128:256]
        A3S = cst[:, 256:384]
        A3V = cst[:, 384:512]
        Cm = cst[:, 512:640]
        KMUS = cst[:, 640:648]
        KVARSE = cst[:, 648:656]
        ZERO = cst[:, 656:657]

        # ACT table warmup: Sqrt selects a table set that also serves
        # Square/Identity -- one load, no switches later.
        warm = cons.tile([1, 1], f32, tag="warm")
        nc.vector.memset(warm[:], 1.0)
        nc.scalar.activation(warm[:], warm[:], AF.Sqrt)

        ymu_prev = ZERO
        yvar_prev = ZERO
        stats = {}

        def phase_a(g):
            """Load + row stats for group g (big DVE/ACT ops, data-gated)."""
            xt = xp.tile([128, FREE], f32, tag="x")
            nc.sync.dma_start(xt[:], x_in[GB * g:GB * g + GB])
            s = sm.tile([128, 1], f32, tag=f"s{g}")
            nc.vector.reduce_sum(s[:], xt[:], axis=X)
            junk = jp.tile([128, FREE], bf16, tag="junk")
            q = sm.tile([128, 1], f32, tag=f"q{g}")
            nc.scalar.activation(junk[:], xt[:], AF.Square, accum_out=q[:])
            stats[g] = (xt, s, q)

        def phase_b(g):
            """EMA chain + normalize + store for group g (chain-gated)."""
            nonlocal ymu_prev, yvar_prev
            xt, s, q = stats.pop(g)

            p_smu = pp.tile([128, 1], f32, tag="psmu")
            nc.tensor.matmul(p_smu[:], AS, s[:], start=True, stop=False)
            nc.tensor.matmul(p_smu[:], Cm, ymu_prev, start=False, stop=True)
            p_ymu = pp.tile([128, 1], f32, tag="pymu")
            nc.tensor.matmul(p_ymu[:], A3S, s[:], start=True, stop=True)

            # whole stale/var chain on DVE (same-engine in-order: no sem
            # hops); ACT only does Sqrt, squares of the big tiles, norms
            mu = sm.tile([128, 1], f32, tag=f"mu{g}")
            nc.vector.tensor_scalar_mul(mu[:], s[:], RN)
            musq = sm.tile([128, 1], f32, tag=f"musq{g}")
            nc.vector.tensor_tensor(out=musq[:], in0=mu[:], in1=mu[:],
                                    op=ALU.mult)
            smu = sm.tile([128, 1], f32, tag=f"smu{g}")
            nc.vector.tensor_tensor(out=smu[:], in0=p_smu[:],
                                    in1=KMUS[:, g:g + 1], op=ALU.add)
            e = sm.tile([128, 1], f32, tag=f"e{g}")
            nc.vector.tensor_tensor(out=e[:], in0=mu[:], in1=smu[:],
                                    op=ALU.subtract)
            e2 = sm.tile([128, 1], f32, tag=f"e2{g}")
            nc.vector.tensor_tensor(out=e2[:], in0=e[:], in1=e[:],
                                    op=ALU.mult)
            vpr = sm.tile([128, 1], f32, tag=f"vpr{g}")
            nc.vector.scalar_tensor_tensor(vpr[:], q[:], RN, musq[:],
                                           op0=ALU.mult, op1=ALU.subtract)
            vc = sm.tile([128, 1], f32, tag=f"vc{g}")
            nc.vector.scalar_tensor_tensor(vc[:], e2[:], float(AFWD), vpr[:],
                                           op0=ALU.mult, op1=ALU.add)
            # carry update after vc: keeps the stale chain tight on DVE
            ymu = sm.tile([128, 1], f32, tag=f"ymu{g}")
            nc.vector.scalar_tensor_tensor(ymu[:], ymu_prev, M4, p_ymu[:],
                                           op0=ALU.mult, op1=ALU.add)

            p_svar = pp.tile([128, 1], f32, tag="psvar")
            nc.tensor.matmul(p_svar[:], AV, vc[:], start=True, stop=False)
            nc.tensor.matmul(p_svar[:], Cm, yvar_prev, start=False, stop=True)
            p_yvar = pp.tile([128, 1], f32, tag="pyvar")
            nc.tensor.matmul(p_yvar[:], A3V, vc[:], start=True, stop=True)

            yvar = sm.tile([128, 1], f32, tag=f"yvar{g}")
            nc.vector.scalar_tensor_tensor(yvar[:], yvar_prev, M4, p_yvar[:],
                                           op0=ALU.mult, op1=ALU.add)
            std = sm.tile([128, 1], f32, tag=f"std{g}")
            nc.scalar.activation(std[:], p_svar[:], AF.Sqrt,
                                 bias=KVARSE[:, g:g + 1])
            Sg = sm.tile([128, 1], f32, tag=f"S{g}")
            nc.vector.reciprocal(Sg[:], std[:])
            Tg = sm.tile([128, 1], f32, tag=f"T{g}")
            nc.vector.scalar_tensor_tensor(Tg[:], smu[:], -1.0, Sg[:],
                                           op0=ALU.mult, op1=ALU.mult)

            # normalize split along the FREE dim (engine time scales with
            # free size, not partitions): DVE half first (T_g lands on
            # DVE, no cross-engine hop), ACT half in parallel; each half
            # stores independently so the DMA engines stay packed.
            outt = op.tile([128, FREE], bf16, tag="out")
            HF = FREE // 2
            nc.vector.tensor_scalar(outt[:, HF:], xt[:, HF:], Sg[:], Tg[:],
                                    op0=ALU.mult, op1=ALU.add)
            nc.gpsimd.dma_start(out_d[GB * g:GB * g + GB, :, HF:],
                                outt[:, HF:])
            nc.scalar.activation(outt[:, :HF], xt[:, :HF], AF.Identity,
                                 bias=Tg[:], scale=Sg[:])
            nc.gpsimd.dma_start(out_d[GB * g:GB * g + GB, :, :HF],
                                outt[:, :HF])

            ymu_prev = ymu[:]
            yvar_prev = yvar[:]

        # software-pipelined emission: group g+1's data-gated stats are
        # queued ahead of group g's chain-gated tail, so the per-engine
        # in-order SEQs never stall a ready reduce/square behind a norm
        phase_a(0)
        for g in range(G):
            if g + 1 < G:
                phase_a(g + 1)
            phase_b(g)

    nc.compile()
    return nc


def _get_module():
    if "nc" not in _CACHE:
        _CACHE["nc"] = _build_module()
    return _CACHE["nc"]


def kernel(x, m, var, m_p, var_p, u, u_p, v_p, beta_p, alpha_p):
    from concourse.bass_utils import run_bass_kernel_spmd

    nc = _get_module()

    x = np.asarray(x, dtype=np.float32)
    m = np.asarray(m, dtype=np.float32)
    var = np.asarray(var, dtype=np.float32)
    m_p = np.asarray(m_p, dtype=np.float32)
    var_p = np.asarray(var_p, dtype=np.float32)

    x4 = x.reshape(B, C, FREE)
    in_maps = []
    for i in range(NCORES):
        cs = slice(i * CSH, (i + 1) * CSH)
        in_maps.append({
            "x": np.ascontiguousarray(x4[:, cs, :]),
            "cst": _build_const_block(m[:, cs], var[:, cs],
                                      m_p[:, cs], var_p[:, cs]),
        })

    res = run_bass_kernel_spmd(nc, in_maps, list(range(NCORES)),
                               **_CACHE.get("run_kwargs", {}))
    _CACHE["last_results"] = res
    out = np.empty((B, C, FREE), dtype=np.float32)
    for i in range(NCORES):
        out[:, i * CSH:(i + 1) * CSH, :] = np.asarray(
            res.results[i]["out"]).astype(np.float32)
    return out.reshape(B, C, H, W)
